# revision 6
# baseline (speedup 1.0000x reference)
"""Trainium2 Bass kernel: ROI-MLP classifier (dense_mlp).

Computation (per the reference):
    xs  = x[:, :128, :]                          # [64, 128, 3136]
    h   = leaky_relu(xs @ W1 + b1, 0.1)          # [64, 128, 64]
    out = leaky_relu(h @ W2 + b2, 0.1)           # [64, 128, 21]
    out[b, r, :] = 0 where r >= min(keep_count[b], 128)

Sharding: pure data parallel over the batch dim across 8 NeuronCores
(8 batches/core); the tiny MLP weights are replicated.

Device plan per core (SPMD, no collectives):
  - DMA x[b, :128, :] (1.6 MB contiguous) per batch; 12.85 MB/core total
    == the HBM roofline for this problem.
  - PE-transpose X row-tiles into feature-major XT tiles (matmul
    contraction must sit on the partition dim).
  - mm1: W1 k-tiles stationary, XT moving -> h.T [64, rows] in PSUM,
    rows grouped GROUP*128 wide so the moving dim is >=256.
  - ScalarE activation: fused bias + leaky-relu (Lrelu, alpha=0.1).
  - mm2: W2 stationary -> out.T [21, rows]; fused bias + leaky-relu.
  - PE-transpose back to [128 rows, 21], multiply by a per-partition
    mask column (iota < keep_count), DMA out.
"""

import os
import sys
from contextlib import ExitStack

import numpy as np

for _p in ("/opt/trn_rl_repo", "/root/.axon_site/_ro/trn_rl_repo"):
    if os.path.isdir(_p) and _p not in sys.path:
        sys.path.insert(0, _p)

import ml_dtypes  # noqa: E402
import concourse.bass as bass  # noqa: E402
import concourse.bacc as bacc  # noqa: E402
import concourse.tile as tile  # noqa: E402
from concourse import mybir  # noqa: E402
from concourse.bass_utils import run_bass_kernel_spmd  # noqa: E402

# Problem constants (hardcoded; kernel.py must be self-contained).
B, R, FEAT = 64, 300, 3136
S = 128          # SELECTED_PROPOSAL: only the first 128 ROIs are computed
HID = 64
NCLS = 21
SLOPE = 0.1
NCORES = 8
BPC = B // NCORES            # batches per core
KT = (FEAT + 127) // 128     # 25 contraction tiles (24x128 + 1x64)
GROUP = 4                    # batches per matmul group -> moving dim 512
NG = BPC // GROUP

# "bf16": cast x/W to bf16 (fp32 accumulate).  "f32r": fp32 bits through
# the PE's fast single-pass fp32 mode.  "f32": exact but 4x slower PE.
MODE = os.environ.get("BASS_CLS_MODE", "bf16")

F32 = mybir.dt.float32
F32R = mybir.dt.float32r
BF16 = mybir.dt.bfloat16
I32 = mybir.dt.int32
LRELU = mybir.ActivationFunctionType.Lrelu


def _build(mode: str) -> bass.Bass:
    xdt = BF16 if mode == "bf16" else F32

    def mm(ap):
        # PE-side dtype: f32r mode reinterprets fp32 tiles for the fast path.
        return ap.bitcast(F32R) if mode == "f32r" else ap

    nc = bacc.Bacc(
        "TRN2",
        target_bir_lowering=False,
        debug=False,
        num_devices=NCORES,
    )
    x_d = nc.declare_dram_parameter("x", [BPC, S, FEAT], F32, isOutput=False)
    kc_d = nc.declare_dram_parameter("kc", [1, BPC], I32, isOutput=False)
    w1_d = nc.declare_dram_parameter("w1", [FEAT, HID], xdt, isOutput=False)
    b1_d = nc.declare_dram_parameter("b1", [HID, 1], F32, isOutput=False)
    w2_d = nc.declare_dram_parameter("w2", [HID, NCLS], xdt, isOutput=False)
    b2_d = nc.declare_dram_parameter("b2", [NCLS, 1], F32, isOutput=False)
    id_d = nc.declare_dram_parameter("ident", [S, S], xdt, isOutput=False)
    id2_d = nc.declare_dram_parameter("ident32", [32, 32], F32, isOutput=False)
    iota_d = nc.declare_dram_parameter("iota", [S, 1], F32, isOutput=False)
    out_d = nc.declare_dram_parameter("out", [BPC, S, NCLS], F32, isOutput=True)

    with ExitStack() as ctx:
        tc = ctx.enter_context(tile.TileContext(nc))
        const = ctx.enter_context(tc.tile_pool(name="const", bufs=1))
        xp = ctx.enter_context(tc.tile_pool(name="xp", bufs=3))
        xtp = ctx.enter_context(tc.tile_pool(name="xtp", bufs=2))
        hsp = ctx.enter_context(tc.tile_pool(name="hsp", bufs=2))
        osp = ctx.enter_context(tc.tile_pool(name="osp", bufs=2))
        outp = ctx.enter_context(tc.tile_pool(name="outp", bufs=3))
        trps = ctx.enter_context(
            tc.tile_pool(name="trps", bufs=3, space=bass.MemorySpace.PSUM)
        )
        hps = ctx.enter_context(
            tc.tile_pool(name="hps", bufs=2, space=bass.MemorySpace.PSUM)
        )
        ops_ = ctx.enter_context(
            tc.tile_pool(name="ops", bufs=1, space=bass.MemorySpace.PSUM)
        )
        otps = ctx.enter_context(
            tc.tile_pool(name="otps", bufs=1, space=bass.MemorySpace.PSUM)
        )

        # ---- constants ----
        w1_sb = const.tile([128, KT * HID], xdt)
        for k in range(KT):
            szk = min(128, FEAT - 128 * k)
            nc.sync.dma_start(
                w1_sb[:szk, k * HID:(k + 1) * HID],
                w1_d[128 * k:128 * k + szk, :],
            )
        w2_sb = const.tile([HID, NCLS], xdt)
        nc.sync.dma_start(w2_sb[:, :], w2_d[:, :])
        b1_sb = const.tile([HID, 1], F32)
        nc.sync.dma_start(b1_sb[:, :], b1_d[:, :])
        b2_sb = const.tile([NCLS, 1], F32)
        nc.sync.dma_start(b2_sb[:, :], b2_d[:, :])
        id_sb = const.tile([S, S], xdt)
        nc.sync.dma_start(id_sb[:, :], id_d[:, :])
        id2_sb = const.tile([32, 32], F32)
        nc.sync.dma_start(id2_sb[:, :], id2_d[:, :])
        iota_sb = const.tile([S, 1], F32)
        nc.sync.dma_start(iota_sb[:, :], iota_d[:, :])

        # mask[r, b] = 1.0 if r < keep_count[b] else 0.0
        kci = const.tile([1, BPC], I32)
        nc.sync.dma_start(kci[:, :], kc_d[:, :])
        kcf = const.tile([1, BPC], F32)
        nc.vector.tensor_copy(kcf[:, :], kci[:, :])
        ones_sb = const.tile([1, S], F32)
        nc.vector.memset(ones_sb[:, :], 1.0)
        kcp = otps.tile([S, BPC], F32)
        nc.tensor.matmul(
            kcp[:, :], ones_sb[:, :], kcf[:, :], start=True, stop=True
        )
        kcb = const.tile([S, BPC], F32)
        nc.vector.tensor_copy(kcb[:, :], kcp[:, :])
        mask = const.tile([S, BPC], F32)
        nc.vector.tensor_scalar(
            mask[:, :], kcb[:, :], iota_sb[:, 0:1], None,
            op0=mybir.AluOpType.is_gt,
        )

        # ---- main loop ----
        NW = GROUP * S
        for g in range(NG):
            xt = xtp.tile([128, KT, NW], xdt)
            for bi in range(GROUP):
                b = g * GROUP + bi
                xb = xp.tile([S, FEAT], xdt)
                if mode == "bf16":
                    nc.gpsimd.dma_start(xb[:, :], x_d[b, :, :])  # casts f32->bf16
                else:
                    nc.sync.dma_start(xb[:, :], x_d[b, :, :])
                for q in range((KT + 3) // 4):
                    kn = min(4, KT - 4 * q)
                    pt = trps.tile([128, 4 * S], xdt)
                    for j in range(kn):
                        k = 4 * q + j
                        szk = min(128, FEAT - 128 * k)
                        nc.tensor.transpose(
                            pt[:szk, j * S:(j + 1) * S],
                            mm(xb[:, 128 * k:128 * k + szk]),
                            mm(id_sb[:, :]),
                        )
                    if kn == 4 and 4 * q + 3 < KT - 1:
                        nc.vector.tensor_copy(
                            xt[:, 4 * q:4 * q + 4, bi * S:(bi + 1) * S],
                            pt.rearrange("p (j r) -> p j r", j=4),
                        )
                    else:
                        for j in range(kn):
                            k = 4 * q + j
                            szk = min(128, FEAT - 128 * k)
                            nc.vector.tensor_copy(
                                xt[:szk, k, bi * S:(bi + 1) * S],
                                pt[:szk, j * S:(j + 1) * S],
                            )

            hp = hps.tile([HID, NW], F32)
            for k in range(KT):
                szk = min(128, FEAT - 128 * k)
                nc.tensor.matmul(
                    hp[:, :],
                    mm(w1_sb[:szk, k * HID:(k + 1) * HID]),
                    mm(xt[:szk, k, :]),
                    start=(k == 0),
                    stop=(k == KT - 1),
                )
            # leaky_relu(z) == max(z, 0.1*z) for slope in (0, 1)
            hz = hsp.tile([HID, NW], F32, tag="hz")
            nc.scalar.activation(
                hz[:, :], hp[:, :], mybir.ActivationFunctionType.Identity,
                bias=b1_sb[:, 0:1], scale=1.0,
            )
            hzs = hsp.tile([HID, NW], F32, tag="hzs")
            nc.vector.tensor_scalar(
                hzs[:, :], hp[:, :], b1_sb[:, 0:1], SLOPE,
                op0=mybir.AluOpType.add, op1=mybir.AluOpType.mult,
            )
            hs = hsp.tile([HID, NW], xdt, tag="hs")
            nc.vector.tensor_max(hs[:, :], hz[:, :], hzs[:, :])
            op2 = ops_.tile([NCLS, NW], F32)
            nc.tensor.matmul(
                op2[:, :], mm(w2_sb[:, :]), mm(hs[:, :]), start=True, stop=True
            )
            oz = osp.tile([NCLS, NW], F32, tag="oz")
            nc.scalar.activation(
                oz[:, :], op2[:, :], mybir.ActivationFunctionType.Identity,
                bias=b2_sb[:, 0:1], scale=1.0,
            )
            ozs = osp.tile([NCLS, NW], F32, tag="ozs")
            nc.vector.tensor_scalar(
                ozs[:, :], op2[:, :], b2_sb[:, 0:1], SLOPE,
                op0=mybir.AluOpType.add, op1=mybir.AluOpType.mult,
            )
            os2 = osp.tile([NCLS, NW], F32, tag="os2")
            nc.vector.tensor_max(os2[:, :], oz[:, :], ozs[:, :])
            for bi in range(GROUP):
                b = g * GROUP + bi
                ot = otps.tile([S, 32], F32)
                nc.tensor.transpose(
                    ot[:, :NCLS],
                    os2[:NCLS, bi * S:(bi + 1) * S],
                    id2_sb[:NCLS, :NCLS],
                )
                ob = outp.tile([S, NCLS], F32)
                nc.vector.tensor_scalar(
                    ob[:, :], ot[:, :NCLS], mask[:, b:b + 1], None,
                    op0=mybir.AluOpType.mult,
                )
                nc.sync.dma_start(out_d[b, :, :], ob[:, :])

    nc.compile()
    return nc


_CACHE: dict = {}


def _program(mode: str) -> bass.Bass:
    if mode not in _CACHE:
        _CACHE[mode] = _build(mode)
    return _CACHE[mode]


def make_in_maps(x, W1, b1, W2, b2, keep_count, mode=None):
    """Shard FULL inputs into per-core input maps (also used by test.py)."""
    mode = mode or MODE
    xdt_np = ml_dtypes.bfloat16 if mode == "bf16" else np.float32
    x = np.asarray(x, np.float32)
    xs = np.ascontiguousarray(x[:, :S, :])
    kc = np.asarray(keep_count, np.int32)
    w1h = np.asarray(W1, np.float32).astype(xdt_np)
    w2h = np.asarray(W2, np.float32).astype(xdt_np)
    b1h = np.asarray(b1, np.float32).reshape(HID, 1)
    b2h = np.asarray(b2, np.float32).reshape(NCLS, 1)
    ident = np.eye(S, dtype=xdt_np)
    id32 = np.eye(32, dtype=np.float32)
    iota = np.arange(S, dtype=np.float32).reshape(S, 1)
    in_maps = []
    for c in range(NCORES):
        in_maps.append({
            "x": xs[c * BPC:(c + 1) * BPC],
            "kc": kc[c * BPC:(c + 1) * BPC].reshape(1, BPC),
            "w1": w1h, "b1": b1h, "w2": w2h, "b2": b2h,
            "ident": ident, "ident32": id32, "iota": iota,
        })
    return in_maps


def kernel(x, W1, b1, W2, b2, keep_count):
    nc = _program(MODE)
    in_maps = make_in_maps(x, W1, b1, W2, b2, keep_count, MODE)
    res = run_bass_kernel_spmd(nc, in_maps, core_ids=list(range(NCORES)))
    outs = [np.asarray(res.results[c]["out"]) for c in range(NCORES)]
    return np.concatenate(outs, axis=0).reshape(B, S, NCLS).astype(np.float32)


# revision 27
# speedup vs baseline: 165.5736x; 165.5736x over previous
"""Trainium2 Bass kernel: ROI-MLP classifier (dense_mlp).

Computation (per the reference):
    xs  = x[:, :128, :]                          # [64, 128, 3136]
    h   = leaky_relu(xs @ W1 + b1, 0.1)          # [64, 128, 64]
    out = leaky_relu(h @ W2 + b2, 0.1)           # [64, 128, 21]
    out[b, r, :] = 0 where r >= min(keep_count[b], 128)

Sharding: pure data parallel over the batch dim across 8 NeuronCores
(8 batches/core); the tiny MLP weights are replicated.

Device plan per core (SPMD, no collectives):
  - DMA x[b, :128, :] (1.6 MB contiguous) per batch; 12.85 MB/core total
    == the HBM roofline for this problem.
  - PE-transpose X row-tiles into feature-major XT tiles (matmul
    contraction must sit on the partition dim).
  - mm1: W1 k-tiles stationary, XT moving -> h.T [64, rows] in PSUM,
    rows grouped GROUP*128 wide so the moving dim is >=256.
  - ScalarE activation: fused bias + leaky-relu (Lrelu, alpha=0.1).
  - mm2: W2 stationary -> out.T [21, rows]; fused bias + leaky-relu.
  - PE-transpose back to [128 rows, 21], multiply by a per-partition
    mask column (iota < keep_count), DMA out.
"""

import os
import sys
from contextlib import ExitStack

import numpy as np

for _p in ("/opt/trn_rl_repo", "/root/.axon_site/_ro/trn_rl_repo"):
    if os.path.isdir(_p) and _p not in sys.path:
        sys.path.insert(0, _p)

import ml_dtypes  # noqa: E402
import concourse.bass as bass  # noqa: E402
import concourse.bacc as bacc  # noqa: E402
import concourse.tile as tile  # noqa: E402
from concourse import mybir  # noqa: E402
from concourse.bass_utils import run_bass_kernel_spmd  # noqa: E402

# Problem constants (hardcoded; kernel.py must be self-contained).
B, R, FEAT = 64, 300, 3136
S = 128          # SELECTED_PROPOSAL: only the first 128 ROIs are computed
HID = 64
NCLS = 21
SLOPE = 0.1
NCORES = 8
BPC = B // NCORES            # batches per core
KT = (FEAT + 127) // 128     # 25 contraction tiles (24x128 + 1x64)
GROUP = 2                    # batches per matmul group -> moving dim 256
NG = BPC // GROUP

# "bf16": cast x/W to bf16 (fp32 accumulate).  "f32r": fp32 bits through
# the PE's fast single-pass fp32 mode.  "f32": exact but 4x slower PE.
MODE = os.environ.get("BASS_CLS_MODE", "bf16")

F32 = mybir.dt.float32
F32R = mybir.dt.float32r
BF16 = mybir.dt.bfloat16
I32 = mybir.dt.int32
LRELU = mybir.ActivationFunctionType.Lrelu


def _build(mode: str, niter: int = 1) -> bass.Bass:
    # xdt: dtype of the transposed activations / weights fed to the PE.
    # x is always loaded as f32 (HWDGE; SWDGE cast-DMA costs 2.4us/load to
    # emit descriptors) and PE-transposed in f32; the PSUM->SBUF copy does
    # the bf16 cast for free.
    xdt = BF16 if mode == "bf16" else F32

    def mm(ap):
        # PE-side dtype: f32r mode reinterprets fp32 tiles for the fast path.
        return ap.bitcast(F32R) if mode == "f32r" else ap

    nc = bacc.Bacc(
        "TRN2",
        target_bir_lowering=False,
        debug=False,
        num_devices=NCORES,
    )
    x_d = nc.declare_dram_parameter("x", [BPC, S, FEAT], F32, isOutput=False)
    kc_d = nc.declare_dram_parameter("kc", [1, BPC], I32, isOutput=False)
    w1_d = nc.declare_dram_parameter("w1", [FEAT, HID], xdt, isOutput=False)
    b1_d = nc.declare_dram_parameter("b1", [HID, 1], F32, isOutput=False)
    w2_d = nc.declare_dram_parameter("w2", [HID, NCLS], xdt, isOutput=False)
    b2_d = nc.declare_dram_parameter("b2", [NCLS, 1], F32, isOutput=False)
    id_d = nc.declare_dram_parameter("ident", [S, S], F32, isOutput=False)
    id2_d = nc.declare_dram_parameter("ident32", [32, 32], F32, isOutput=False)
    iota_d = nc.declare_dram_parameter("iota", [S, 1], F32, isOutput=False)
    out_d = nc.declare_dram_parameter("out", [BPC, S, NCLS], F32, isOutput=True)

    with ExitStack() as ctx:
        tc = ctx.enter_context(tile.TileContext(nc))
        const = ctx.enter_context(tc.tile_pool(name="const", bufs=1))
        xp = ctx.enter_context(tc.tile_pool(name="xp", bufs=5))
        xtp = ctx.enter_context(tc.tile_pool(name="xtp", bufs=3))
        hsp = ctx.enter_context(tc.tile_pool(name="hsp", bufs=2))
        osp = ctx.enter_context(tc.tile_pool(name="osp", bufs=2))
        outp = ctx.enter_context(tc.tile_pool(name="outp", bufs=3))
        trps = ctx.enter_context(
            tc.tile_pool(name="trps", bufs=3, space=bass.MemorySpace.PSUM)
        )
        hps = ctx.enter_context(
            tc.tile_pool(name="hps", bufs=2, space=bass.MemorySpace.PSUM)
        )
        ops_ = ctx.enter_context(
            tc.tile_pool(name="ops", bufs=1, space=bass.MemorySpace.PSUM)
        )
        otps = ctx.enter_context(
            tc.tile_pool(name="otps", bufs=2, space=bass.MemorySpace.PSUM)
        )

        # ---- x loads first: they are the critical DMA stream, so they own
        # the SP HWDGE queue; constants/stores go via the ACT HWDGE queue.
        all_xbs = []
        H2 = FEAT // 2
        for b in range(BPC):
            xb = xp.tile([S, FEAT], F32)
            nc.sync.dma_start(xb[:, :H2], x_d[b, :, :H2])
            nc.sync.dma_start(xb[:, H2:], x_d[b, :, H2:])
            all_xbs.append(xb)

        # ---- constants (ACT HWDGE queue; identity first — transposes need it)
        id_sb = const.tile([S, S], F32)
        nc.scalar.dma_start(id_sb[:, :], id_d[:, :])
        w1_sb = const.tile([128, KT * HID], xdt)
        kfull = FEAT // 128  # 24 full k-tiles, one 64-row tail
        nc.scalar.dma_start(
            w1_sb[:, :kfull * HID].rearrange("p (k h) -> p k h", k=kfull),
            w1_d[:kfull * 128, :].rearrange("(k p) h -> p k h", p=128),
        )
        nc.scalar.dma_start(
            w1_sb[:FEAT - kfull * 128, kfull * HID:],
            w1_d[kfull * 128:, :],
        )
        w2_sb = const.tile([HID, NCLS], xdt)
        nc.scalar.dma_start(w2_sb[:, :], w2_d[:, :])
        b1_sb = const.tile([HID, 1], F32)
        nc.scalar.dma_start(b1_sb[:, :], b1_d[:, :])
        b2_sb = const.tile([NCLS, 1], F32)
        nc.scalar.dma_start(b2_sb[:, :], b2_d[:, :])
        id2_sb = const.tile([32, 32], F32)
        nc.scalar.dma_start(id2_sb[:, :], id2_d[:, :])
        iota_sb = const.tile([S, 1], F32)
        nc.scalar.dma_start(iota_sb[:, :], iota_d[:, :])

        # mask[r, b] = 1.0 if r < keep_count[b] else 0.0
        kci = const.tile([1, BPC], I32)
        nc.scalar.dma_start(kci[:, :], kc_d[:, :])
        kcf = const.tile([1, BPC], F32)
        nc.vector.tensor_copy(kcf[:, :], kci[:, :])
        ones_sb = const.tile([1, S], F32)
        nc.vector.memset(ones_sb[:, :], 1.0)
        kcp = otps.tile([S, 32], F32, tag="ot")
        nc.tensor.matmul(
            kcp[:, :BPC], ones_sb[:, :], kcf[:, :], start=True, stop=True
        )
        kcb = const.tile([S, BPC], F32)
        nc.vector.tensor_copy(kcb[:, :], kcp[:, :BPC])
        mask = const.tile([S, BPC], F32)
        nc.vector.tensor_scalar(
            mask[:, :], kcb[:, :], iota_sb[:, 0:1], None,
            op0=mybir.AluOpType.is_gt,
        )

        # ---- main loop: each batch is an independent pipeline ----
        for it in range(niter):
            if it > 0:
                all_xbs = []
                for b in range(BPC):
                    xb = xp.tile([S, FEAT], F32)
                    nc.sync.dma_start(xb[:, :H2], x_d[b, :, :H2])
                    nc.sync.dma_start(xb[:, H2:], x_d[b, :, H2:])
                    all_xbs.append(xb)
            for b in range(BPC):
                xb = all_xbs[b]
                xt = xtp.tile([128, KT, S], xdt)
                for q in range((KT + 3) // 4):
                    kn = min(4, KT - 4 * q)
                    pt = trps.tile([128, 4 * S], F32)
                    for j in range(kn):
                        k = 4 * q + j
                        szk = min(128, FEAT - 128 * k)
                        nc.tensor.transpose(
                            pt[:szk, j * S:(j + 1) * S],
                            mm(xb[:, 128 * k:128 * k + szk]),
                            mm(id_sb[:, :]),
                        )
                    if kn == 4:
                        dst = xt[:, 4 * q:4 * q + 4, :]
                        src = pt.rearrange("p (j r) -> p j r", j=4)
                        if q in (2, 5):
                            # offload ~2/7 of PSUM->SBUF traffic to ScalarE
                            # (Identity reuses the warmed bias-op table)
                            nc.scalar.activation(
                                dst, src,
                                mybir.ActivationFunctionType.Identity,
                                bias=0.0, scale=1.0,
                            )
                        else:
                            nc.vector.tensor_copy(dst, src)
                    else:
                        for j in range(kn):
                            k = 4 * q + j
                            szk = min(128, FEAT - 128 * k)
                            nc.vector.tensor_copy(
                                xt[:szk, k, :],
                                pt[:szk, j * S:(j + 1) * S],
                            )

                hp = hps.tile([HID, S], F32)
                for k in range(KT):
                    szk = min(128, FEAT - 128 * k)
                    nc.tensor.matmul(
                        hp[:, :],
                        mm(w1_sb[:szk, k * HID:(k + 1) * HID]),
                        mm(xt[:szk, k, :]),
                        start=(k == 0),
                        stop=(k == KT - 1),
                    )
                # leaky_relu(z) == max(z, 0.1*z) for slope in (0, 1)
                hz = hsp.tile([HID, S], F32, tag="hz")
                nc.scalar.activation(
                    hz[:, :], hp[:, :], mybir.ActivationFunctionType.Identity,
                    bias=b1_sb[:, 0:1], scale=1.0,
                )
                hzs = hsp.tile([HID, S], F32, tag="hzs")
                nc.vector.tensor_scalar(
                    hzs[:, :], hp[:, :], b1_sb[:, 0:1], SLOPE,
                    op0=mybir.AluOpType.add, op1=mybir.AluOpType.mult,
                )
                hs = hsp.tile([HID, S], xdt, tag="hs")
                nc.vector.tensor_max(hs[:, :], hz[:, :], hzs[:, :])
                op2 = ops_.tile([NCLS, S], F32)
                nc.tensor.matmul(
                    op2[:, :], mm(w2_sb[:, :]), mm(hs[:, :]),
                    start=True, stop=True,
                )
                oz = osp.tile([NCLS, S], F32, tag="oz")
                nc.scalar.activation(
                    oz[:, :], op2[:, :], mybir.ActivationFunctionType.Identity,
                    bias=b2_sb[:, 0:1], scale=1.0,
                )
                ozs = osp.tile([NCLS, S], F32, tag="ozs")
                nc.vector.tensor_scalar(
                    ozs[:, :], op2[:, :], b2_sb[:, 0:1], SLOPE,
                    op0=mybir.AluOpType.add, op1=mybir.AluOpType.mult,
                )
                os2 = osp.tile([NCLS, S], F32, tag="os2")
                nc.vector.tensor_max(os2[:, :], oz[:, :], ozs[:, :])
                ot = otps.tile([S, 32], F32, tag="ot")
                nc.tensor.transpose(
                    ot[:, :NCLS], os2[:NCLS, :], id2_sb[:NCLS, :NCLS]
                )
                ob = outp.tile([S, NCLS], F32)
                nc.vector.tensor_scalar(
                    ob[:, :], ot[:, :NCLS], mask[:, b:b + 1], None,
                    op0=mybir.AluOpType.mult,
                )
                nc.scalar.dma_start(out_d[b, :, :], ob[:, :])

    nc.compile()
    return nc


_CACHE: dict = {}


def _program(mode: str, niter: int = 1) -> bass.Bass:
    key = (mode, niter)
    if key not in _CACHE:
        _CACHE[key] = _build(mode, niter)
    return _CACHE[key]


def make_in_maps(x, W1, b1, W2, b2, keep_count, mode=None):
    """Shard FULL inputs into per-core input maps (also used by test.py)."""
    mode = mode or MODE
    xdt_np = ml_dtypes.bfloat16 if mode == "bf16" else np.float32
    x = np.asarray(x, np.float32)
    xs = np.ascontiguousarray(x[:, :S, :])
    kc = np.asarray(keep_count, np.int32)
    w1h = np.asarray(W1, np.float32).astype(xdt_np)
    w2h = np.asarray(W2, np.float32).astype(xdt_np)
    b1h = np.asarray(b1, np.float32).reshape(HID, 1)
    b2h = np.asarray(b2, np.float32).reshape(NCLS, 1)
    ident = np.eye(S, dtype=np.float32)
    id32 = np.eye(32, dtype=np.float32)
    iota = np.arange(S, dtype=np.float32).reshape(S, 1)
    in_maps = []
    for c in range(NCORES):
        in_maps.append({
            "x": xs[c * BPC:(c + 1) * BPC],
            "kc": kc[c * BPC:(c + 1) * BPC].reshape(1, BPC),
            "w1": w1h, "b1": b1h, "w2": w2h, "b2": b2h,
            "ident": ident, "ident32": id32, "iota": iota,
        })
    return in_maps


def kernel(x, W1, b1, W2, b2, keep_count):
    nc = _program(MODE)
    in_maps = make_in_maps(x, W1, b1, W2, b2, keep_count, MODE)
    res = run_bass_kernel_spmd(nc, in_maps, core_ids=list(range(NCORES)))
    outs = [np.asarray(res.results[c]["out"]) for c in range(NCORES)]
    return np.concatenate(outs, axis=0).reshape(B, S, NCLS).astype(np.float32)
